# revision 6
# baseline (speedup 1.0000x reference)
"""Trainium2 Bass kernel for CustomMHA (B=4, S=2048, D=1024, H=16).

Sharding: 8 cores = 4 batches x 2 head-groups. Core c handles batch c//2,
heads (c%2)*8 .. (c%2)*8+7. Each core computes its heads' QKV projection,
attention, and a partial output projection (its heads' columns of W_o);
the host sums the two partial Y's per batch.

Per-core structure (bf16 matmuls, fp32 PSUM accumulation):
  - x^T [1024, 2048] resident in SBUF; Q^T/K^T per head-pair as
    [dout, token] tiles (two heads on partition halves 0-63 / 64-127),
    V as [token, head, dh+1] with a ones column for the denominator.
  - scores S^T[k, q] per 128-k tile; the two heads of a pair are packed
    into PE row groups (dh=64 contraction at partition base 0 and 64)
    writing the two halves of one [128, 1024] PSUM tile.
  - softmax: exp on ScalarE with 1/sqrt(d_h) folded into the activation
    scale; no max-subtraction (|scores|/8 stays < ~5).
  - AV: lhsT = [V_h | 1] (M=65), so PSUM row 64 accumulates the softmax
    denominator for free. AV matmuls trail the exp by 2 k-tiles so their
    LDWEIGHTS is never gated on the exp semaphore.
  - normalization: reciprocal_approx_fast + gpsimd partition_broadcast +
    DVE multiply (fused with the PSUM->SBUF move). Head b bounces through
    a [64,512] tile + SBUF->SBUF DMA to reach partitions 64-127.
  - projection: Y[token, e] accumulated over the 4 pair-chunks.
Emission interleaves QKV pairs with attention pairs so the PE fills the
ScalarE-bound attention phase with projection work.
"""

import os
import numpy as np
import ml_dtypes

B, S, D, H, DH = 4, 2048, 1024, 16, 64
NCORES = 8
P = 128

_cache = {}


def _build():
    import concourse.bacc as bacc
    import concourse.tile as tile
    from concourse import mybir

    f32 = mybir.dt.float32
    bf16 = mybir.dt.bfloat16
    Exp = mybir.ActivationFunctionType.Exp

    nc = bacc.Bacc("TRN2", target_bir_lowering=False, debug=False)
    xT = nc.dram_tensor("xT", [D, S], bf16, kind="ExternalInput")
    # wqkp: [d, pair, 256] pair-major (cols 0-127 Q-dout, 128-255 K-dout)
    wqkp = nc.dram_tensor("wqkp", [D, 4, 256], bf16, kind="ExternalInput")
    wv = nc.dram_tensor("wv", [D, 512], bf16, kind="ExternalInput")
    wo = nc.dram_tensor("wo", [512, D], bf16, kind="ExternalInput")
    y = nc.dram_tensor("y", [S, D], f32, kind="ExternalOutput")

    with tile.TileContext(nc) as tc:
        import contextlib
        stack = contextlib.ExitStack()
        with stack:
            sb = stack.enter_context(tc.tile_pool(name="sb", bufs=1))
            ptp = stack.enter_context(tc.tile_pool(name="ptp", bufs=18))
            nrm = stack.enter_context(tc.tile_pool(name="nrm", bufs=4))
            otbp = stack.enter_context(tc.tile_pool(name="otb", bufs=4))
            yp = stack.enter_context(tc.tile_pool(name="yp", bufs=2))
            # PSUM: scores 2x[128,1024] (8KB) + AV 2x[65,512] (4KB) +
            # qkv 2x[128,512] (4KB, reused by proj after close) = 16KB
            psS = stack.enter_context(tc.tile_pool(name="psS", bufs=2, space="PSUM"))
            psO = stack.enter_context(tc.tile_pool(name="psO", bufs=1, space="PSUM"))

            qts = [sb.tile([P, S], bf16, tag=f"qt{p}", name=f"qt{p}") for p in range(4)]
            kts = [sb.tile([P, S], bf16, tag=f"kt{p}", name=f"kt{p}") for p in range(4)]
            ots = [sb.tile([P, S], bf16, tag=f"ot{p}", name=f"ot{p}") for p in range(4)]
            vts = [sb.tile([P, 16, 2, 65], bf16, tag=f"vt{p}", name=f"vt{p}") for p in range(4)]
            wo_sb = sb.tile([P, 4, D], bf16)
            x_sb = sb.tile([P, 8, S], bf16)
            wqk_sb = sb.tile([P, 8, 4, 256], bf16)
            wv_sb = sb.tile([P, 8, 512], bf16)

            # input DMAs, finest-first for fast rampup
            for c in range(8):
                nc.sync.dma_start(out=x_sb[:, c, :], in_=xT[c * P:(c + 1) * P, :])
            nc.sync.dma_start(out=wqk_sb[:], in_=wqkp.ap().rearrange("(c p) j e -> p c j e", p=P))
            nc.sync.dma_start(out=wv_sb[:], in_=wv.ap().rearrange("(c p) e -> p c e", p=P))
            nc.sync.dma_start(out=wo_sb[:], in_=wo.ap().rearrange("(c p) e -> p c e", p=P))
            for p in range(4):
                nc.vector.memset(vts[p][:, :, :, 64:65], 1.0)

            def emit_qk(hp, pool):
                for half, dst in ((0, qts[hp]), (1, kts[hp])):
                    for tb in range(4):
                        ps = pool.tile([P, 512], f32, tag="ps")
                        for c in range(8):
                            nc.tensor.matmul(
                                ps[:],
                                lhsT=wqk_sb[:, c, hp, half * 128:(half + 1) * 128],
                                rhs=x_sb[:, c, tb * 512:(tb + 1) * 512],
                                start=(c == 0), stop=(c == 7),
                            )
                        nc.vector.tensor_copy(dst[:, tb * 512:(tb + 1) * 512], ps[:])

            def emit_v(vh, pool):
                # V for pairs (2*vh, 2*vh+1): rhs cols vh*256..vh*256+255
                for t in range(16):
                    ps = pool.tile([P, 256], f32, tag="ps", name="ps")
                    for c in range(8):
                        nc.tensor.matmul(
                            ps[:],
                            lhsT=x_sb[:, c, t * 128:(t + 1) * 128],
                            rhs=wv_sb[:, c, vh * 256:(vh + 1) * 256],
                            start=(c == 0), stop=(c == 7),
                        )
                    for k in range(2):
                        nc.vector.tensor_copy(
                            vts[2 * vh + k][:, t, :, 0:64],
                            ps[:, k * 128:(k + 1) * 128].rearrange(
                                "p (h d) -> p h d", d=64))

            def emit_attn(hp):
                qt, kt, vt, ot = qts[hp], kts[hp], vts[hp], ots[hp]
                for qb in range(4):
                    qsl = slice(qb * 512, (qb + 1) * 512)
                    oa = psO.tile([65, 512], f32, tag="oa")
                    ob = psO.tile([65, 512], f32, tag="ob")
                    pts = [None] * 16

                    def emit_av(kti):
                        nc.tensor.matmul(
                            oa[:], lhsT=vt[:, kti, 0, :],
                            rhs=pts[kti][:, 0:512],
                            start=(kti == 0), stop=(kti == 15))
                        nc.tensor.matmul(
                            ob[:], lhsT=vt[:, kti, 1, :],
                            rhs=pts[kti][:, 512:1024],
                            start=(kti == 0), stop=(kti == 15))

                    for kti in range(16):
                        ksl = slice(kti * 128, (kti + 1) * 128)
                        s = psS.tile([P, 1024], f32, tag="s")
                        nc.tensor.matmul(
                            s[:, 0:512], lhsT=kt[0:64, ksl], rhs=qt[0:64, qsl],
                            start=True, stop=True)
                        nc.tensor.matmul(
                            s[:, 512:1024], lhsT=kt[64:128, ksl], rhs=qt[64:128, qsl],
                            start=True, stop=True)
                        pt = ptp.tile([P, 1024], bf16, tag="pt")
                        pts[kti] = pt
                        nc.scalar.activation(pt[:], s[:], Exp, scale=0.125)
                        if kti >= 2:
                            emit_av(kti - 2)
                    emit_av(14)
                    emit_av(15)
                    # normalize head a (lanes aligned: psum 0-63 -> ot 0-63)
                    rca = nrm.tile([1, 512], f32, tag="rca")
                    nc.vector.reciprocal(rca[:], oa[64:65, :])
                    bca = nrm.tile([64, 512], f32, tag="bca")
                    nc.gpsimd.partition_broadcast(bca[:], rca[:])
                    nc.vector.tensor_mul(ot[0:64, qsl], oa[0:64, :], bca[:])
                    # head b: normalize at 0-63, DMA-bounce to partitions 64-127
                    rcb = nrm.tile([1, 512], f32, tag="rcb")
                    nc.vector.reciprocal(rcb[:], ob[64:65, :])
                    bcb = nrm.tile([64, 512], f32, tag="bcb")
                    nc.gpsimd.partition_broadcast(bcb[:], rcb[:])
                    otb = otbp.tile([64, 512], bf16, tag="otb")
                    nc.vector.tensor_mul(otb[:], ob[0:64, :], bcb[:])
                    nc.sync.dma_start(out=ot[64:128, qsl], in_=otb[:])

            with tc.tile_pool(name="psA", bufs=2, space="PSUM") as psA:
                emit_qk(0, psA)
                emit_v(0, psA)
                emit_qk(1, psA)
                emit_attn(0)
                emit_v(1, psA)
                emit_qk(2, psA)
                emit_attn(1)
                emit_qk(3, psA)
                emit_attn(2)
                emit_attn(3)

            # ---- output projection (PSUM reuses psA's banks) ----
            with tc.tile_pool(name="psC", bufs=2, space="PSUM") as psC:
                for t in range(16):
                    for eh in range(2):
                        ps = psC.tile([P, 512], f32, tag="psy", name="ps")
                        for c in range(4):
                            nc.tensor.matmul(
                                ps[:],
                                lhsT=ots[c][:, t * 128:(t + 1) * 128],
                                rhs=wo_sb[:, c, eh * 512:(eh + 1) * 512],
                                start=(c == 0), stop=(c == 3),
                            )
                        ysb = yp.tile([P, 512], f32, tag="ysb")
                        nc.scalar.copy(ysb[:], ps[:])
                        nc.sync.dma_start(
                            out=y[t * 128:(t + 1) * 128, eh * 512:(eh + 1) * 512],
                            in_=ysb[:])

    nc.compile()
    return nc


def _get_nc():
    if "nc" not in _cache:
        _cache["nc"] = _build()
    return _cache["nc"]


def make_in_maps(x, W_qkv, W_o):
    bf = ml_dtypes.bfloat16
    in_maps = []
    for c in range(NCORES):
        b, g = c // 2, c % 2
        ds = g * 512  # this core's slice of the head-major model dim
        xTc = np.ascontiguousarray(x[b].T.astype(bf))
        wq = W_qkv[ds:ds + 512, :].reshape(4, 128, D)
        wk = W_qkv[1024 + ds:1024 + ds + 512, :].reshape(4, 128, D)
        # [d, pair, 256]: per pair, 128 Q-dout cols then 128 K-dout cols
        wqkc = np.concatenate([wq, wk], axis=1)      # (4, 256, D)
        wqkc = np.ascontiguousarray(
            wqkc.transpose(2, 0, 1).astype(bf))      # (D, 4, 256)
        wvT = np.ascontiguousarray(
            W_qkv[2048 + ds:2048 + ds + 512, :].T.astype(bf))
        woT = np.ascontiguousarray(W_o[:, ds:ds + 512].T.astype(bf))
        in_maps.append({"xT": xTc, "wqkp": wqkc, "wv": wvT, "wo": woT})
    return in_maps


def kernel(x, W_qkv, W_o):
    from concourse.bass_utils import run_bass_kernel_spmd

    nc = _get_nc()
    in_maps = make_in_maps(np.asarray(x, dtype=np.float32),
                           np.asarray(W_qkv, dtype=np.float32),
                           np.asarray(W_o, dtype=np.float32))
    trace = os.environ.get("KERNEL_TRACE", "") == "1"
    res = run_bass_kernel_spmd(nc, in_maps, core_ids=list(range(NCORES)),
                               trace=trace)
    _cache["last_result"] = res
    Y = np.empty((B, S, D), np.float32)
    for b in range(B):
        Y[b] = res.results[2 * b]["y"] + res.results[2 * b + 1]["y"]
    return Y


# revision 8
# speedup vs baseline: 1.1701x; 1.1701x over previous
"""Trainium2 Bass kernel for CustomMHA (B=4, S=2048, D=1024, H=16).

Sharding: 8 cores = 4 batches x 2 head-groups. Core c handles batch c//2,
heads (c%2)*8 .. (c%2)*8+7. Each core computes its heads' QKV projection,
attention, and a partial output projection (its heads' columns of W_o);
the host sums the two partial Y's per batch.

Per-core structure (bf16 matmuls, fp32 PSUM accumulation):
  - x^T [1024, 2048] resident in SBUF; Q^T/K^T per head-pair as
    [dout, token] tiles (two heads on partition halves 0-63 / 64-127),
    V as [token, head, dh+1] with a ones column for the denominator.
  - scores S^T[k, q] per 128-k tile; the two heads of a pair are packed
    into PE row groups (dh=64 contraction at partition base 0 and 64)
    writing the two halves of one [128, 1024] PSUM tile.
  - softmax: exp on ScalarE with 1/sqrt(d_h) folded into the activation
    scale; no max-subtraction (|scores|/8 stays < ~5).
  - AV: lhsT = [V_h | 1] (M=65), so PSUM row 64 accumulates the softmax
    denominator for free. AV matmuls trail the exp by 2 k-tiles so their
    LDWEIGHTS is never gated on the exp semaphore.
  - normalization: reciprocal_approx_fast + gpsimd partition_broadcast +
    DVE multiply (fused with the PSUM->SBUF move). Head b bounces through
    a [64,512] tile + SBUF->SBUF DMA to reach partitions 64-127.
  - projection: Y[token, e] accumulated over the 4 pair-chunks.
Emission interleaves QKV pairs with attention pairs so the PE fills the
ScalarE-bound attention phase with projection work.
"""

import os
import numpy as np
import ml_dtypes

B, S, D, H, DH = 4, 2048, 1024, 16, 64
NCORES = 8
P = 128

_cache = {}


def _build():
    import concourse.bacc as bacc
    import concourse.tile as tile
    from concourse import mybir

    f32 = mybir.dt.float32
    bf16 = mybir.dt.bfloat16
    Exp = mybir.ActivationFunctionType.Exp

    nc = bacc.Bacc("TRN2", target_bir_lowering=False, debug=False)
    xT = nc.dram_tensor("xT", [D, S], bf16, kind="ExternalInput")
    # wqkp: [d, pair, 256] pair-major (cols 0-127 Q-dout, 128-255 K-dout)
    wqkp = nc.dram_tensor("wqkp", [D, 4, 256], bf16, kind="ExternalInput")
    wv = nc.dram_tensor("wv", [D, 512], bf16, kind="ExternalInput")
    wo = nc.dram_tensor("wo", [512, D], bf16, kind="ExternalInput")
    y = nc.dram_tensor("y", [S, D], f32, kind="ExternalOutput")

    with tile.TileContext(nc) as tc:
        import contextlib
        stack = contextlib.ExitStack()
        with stack:
            sb = stack.enter_context(tc.tile_pool(name="sb", bufs=1))
            ptp = stack.enter_context(tc.tile_pool(name="ptp", bufs=18))
            nrm = stack.enter_context(tc.tile_pool(name="nrm", bufs=2))
            otbp = stack.enter_context(tc.tile_pool(name="otb", bufs=4))
            yp = stack.enter_context(tc.tile_pool(name="yp", bufs=2))
            # PSUM: scores 2x[128,1024] (8KB) + AV 2x[65,512] (4KB) +
            # qkv 2x[128,512] (4KB, reused by proj after close) = 16KB
            psS = stack.enter_context(tc.tile_pool(name="psS", bufs=2, space="PSUM"))
            psO = stack.enter_context(tc.tile_pool(name="psO", bufs=1, space="PSUM"))

            qts = [sb.tile([P, S], bf16, tag=f"qt{p}", name=f"qt{p}") for p in range(4)]
            kts = [sb.tile([P, S], bf16, tag=f"kt{p}", name=f"kt{p}") for p in range(4)]
            ots = [[sb.tile([P, 512], bf16, tag=f"ot{p}_{q}", name=f"ot{p}_{q}")
                    for q in range(4)] for p in range(4)]
            vts = [sb.tile([P, 16, 2, 65], bf16, tag=f"vt{p}", name=f"vt{p}") for p in range(4)]
            wo_sb = sb.tile([P, 4, D], bf16)
            x_sb = sb.tile([P, 8, S], bf16)
            wqk_sb = sb.tile([P, 8, 4, 256], bf16)
            wv_sb = sb.tile([P, 8, 512], bf16)

            # input DMAs, finest-first for fast rampup
            for c in range(8):
                nc.sync.dma_start(out=x_sb[:, c, :], in_=xT[c * P:(c + 1) * P, :])
            nc.sync.dma_start(out=wqk_sb[:], in_=wqkp.ap().rearrange("(c p) j e -> p c j e", p=P))
            nc.sync.dma_start(out=wv_sb[:], in_=wv.ap().rearrange("(c p) e -> p c e", p=P))
            nc.sync.dma_start(out=wo_sb[:], in_=wo.ap().rearrange("(c p) e -> p c e", p=P))
            for p in range(4):
                nc.vector.memset(vts[p][:, :, :, 64:65], 1.0)

            def emit_qk(hp, pool):
                for half, dst in ((0, qts[hp]), (1, kts[hp])):
                    for tb in range(4):
                        ps = pool.tile([P, 512], f32, tag="ps")
                        for c in range(8):
                            nc.tensor.matmul(
                                ps[:],
                                lhsT=wqk_sb[:, c, hp, half * 128:(half + 1) * 128],
                                rhs=x_sb[:, c, tb * 512:(tb + 1) * 512],
                                start=(c == 0), stop=(c == 7),
                            )
                        nc.vector.tensor_copy(dst[:, tb * 512:(tb + 1) * 512], ps[:])

            def emit_v(vh, pool):
                # V for pairs (2*vh, 2*vh+1): rhs cols vh*256..vh*256+255
                for t in range(16):
                    ps = pool.tile([P, 256], f32, tag="ps", name="ps")
                    for c in range(8):
                        nc.tensor.matmul(
                            ps[:],
                            lhsT=x_sb[:, c, t * 128:(t + 1) * 128],
                            rhs=wv_sb[:, c, vh * 256:(vh + 1) * 256],
                            start=(c == 0), stop=(c == 7),
                        )
                    for k in range(2):
                        nc.vector.tensor_copy(
                            vts[2 * vh + k][:, t, :, 0:64],
                            ps[:, k * 128:(k + 1) * 128].rearrange(
                                "p (h d) -> p h d", d=64))

            def emit_attn(hp):
                qt, kt, vt = qts[hp], kts[hp], vts[hp]
                for qb in range(4):
                    qsl = slice(qb * 512, (qb + 1) * 512)
                    ot = ots[hp][qb]
                    oa = psO.tile([65, 512], f32, tag="oa")
                    ob = psO.tile([65, 512], f32, tag="ob")
                    pts = [None] * 16

                    def emit_av(kti):
                        nc.tensor.matmul(
                            oa[:], lhsT=vt[:, kti, 0, :],
                            rhs=pts[kti][:, 0:512],
                            start=(kti == 0), stop=(kti == 15))
                        nc.tensor.matmul(
                            ob[:], lhsT=vt[:, kti, 1, :],
                            rhs=pts[kti][:, 512:1024],
                            start=(kti == 0), stop=(kti == 15))

                    for kti in range(16):
                        ksl = slice(kti * 128, (kti + 1) * 128)
                        s = psS.tile([P, 1024], f32, tag="s")
                        nc.tensor.matmul(
                            s[:, 0:512], lhsT=kt[0:64, ksl], rhs=qt[0:64, qsl],
                            start=True, stop=True)
                        nc.tensor.matmul(
                            s[:, 512:1024], lhsT=kt[64:128, ksl], rhs=qt[64:128, qsl],
                            start=True, stop=True)
                        pt = ptp.tile([P, 1024], bf16, tag="pt")
                        pts[kti] = pt
                        nc.scalar.activation(pt[:], s[:], Exp, scale=0.125)
                        if kti >= 4:
                            emit_av(kti - 4)
                    for kti in (12, 13, 14, 15):
                        emit_av(kti)
                    # free the AV psum banks fast: copy to SBUF, then normalize
                    osa = nrm.tile([65, 512], f32, tag="osa")
                    nc.vector.tensor_copy(osa[:], oa[:])
                    osb = nrm.tile([65, 512], f32, tag="osb")
                    nc.vector.tensor_copy(osb[:], ob[:])
                    # normalize head a (lanes aligned 0-63)
                    rca = nrm.tile([1, 512], f32, tag="rca")
                    nc.vector.reciprocal(rca[:], osa[64:65, :])
                    bca = nrm.tile([64, 512], f32, tag="bca")
                    nc.gpsimd.partition_broadcast(bca[:], rca[:])
                    nc.vector.tensor_mul(ot[0:64, :], osa[0:64, :], bca[:])
                    # head b: normalize at 0-63, DMA-bounce to partitions 64-127
                    rcb = nrm.tile([1, 512], f32, tag="rcb")
                    nc.vector.reciprocal(rcb[:], osb[64:65, :])
                    bcb = nrm.tile([64, 512], f32, tag="bcb")
                    nc.gpsimd.partition_broadcast(bcb[:], rcb[:])
                    otb = otbp.tile([64, 512], bf16, tag="otb")
                    nc.vector.tensor_mul(otb[:], osb[0:64, :], bcb[:])
                    nc.sync.dma_start(out=ot[64:128, :], in_=otb[:])

            with tc.tile_pool(name="psA", bufs=2, space="PSUM") as psA:
                emit_qk(0, psA)
                emit_v(0, psA)
                emit_attn(0)
                emit_qk(1, psA)
                emit_v(1, psA)
                emit_qk(2, psA)
                emit_attn(1)
                emit_qk(3, psA)
                emit_attn(2)
                emit_attn(3)

            # ---- output projection (PSUM reuses psA's banks) ----
            with tc.tile_pool(name="psC", bufs=2, space="PSUM") as psC:
                for t in range(16):
                    for eh in range(2):
                        ps = psC.tile([P, 512], f32, tag="psy", name="ps")
                        tq, tc = t // 4, (t % 4) * 128
                        for c in range(4):
                            nc.tensor.matmul(
                                ps[:],
                                lhsT=ots[c][tq][:, tc:tc + 128],
                                rhs=wo_sb[:, c, eh * 512:(eh + 1) * 512],
                                start=(c == 0), stop=(c == 3),
                            )
                        ysb = yp.tile([P, 512], f32, tag="ysb")
                        nc.scalar.copy(ysb[:], ps[:])
                        nc.sync.dma_start(
                            out=y[t * 128:(t + 1) * 128, eh * 512:(eh + 1) * 512],
                            in_=ysb[:])

    nc.compile()
    return nc


def _get_nc():
    if "nc" not in _cache:
        _cache["nc"] = _build()
    return _cache["nc"]


def make_in_maps(x, W_qkv, W_o):
    bf = ml_dtypes.bfloat16
    in_maps = []
    for c in range(NCORES):
        b, g = c // 2, c % 2
        ds = g * 512  # this core's slice of the head-major model dim
        xTc = np.ascontiguousarray(x[b].T.astype(bf))
        wq = W_qkv[ds:ds + 512, :].reshape(4, 128, D)
        wk = W_qkv[1024 + ds:1024 + ds + 512, :].reshape(4, 128, D)
        # [d, pair, 256]: per pair, 128 Q-dout cols then 128 K-dout cols
        wqkc = np.concatenate([wq, wk], axis=1)      # (4, 256, D)
        wqkc = np.ascontiguousarray(
            wqkc.transpose(2, 0, 1).astype(bf))      # (D, 4, 256)
        wvT = np.ascontiguousarray(
            W_qkv[2048 + ds:2048 + ds + 512, :].T.astype(bf))
        woT = np.ascontiguousarray(W_o[:, ds:ds + 512].T.astype(bf))
        in_maps.append({"xT": xTc, "wqkp": wqkc, "wv": wvT, "wo": woT})
    return in_maps


def kernel(x, W_qkv, W_o):
    from concourse.bass_utils import run_bass_kernel_spmd

    nc = _get_nc()
    in_maps = make_in_maps(np.asarray(x, dtype=np.float32),
                           np.asarray(W_qkv, dtype=np.float32),
                           np.asarray(W_o, dtype=np.float32))
    trace = os.environ.get("KERNEL_TRACE", "") == "1"
    res = run_bass_kernel_spmd(nc, in_maps, core_ids=list(range(NCORES)),
                               trace=trace)
    _cache["last_result"] = res
    Y = np.empty((B, S, D), np.float32)
    for b in range(B):
        Y[b] = res.results[2 * b]["y"] + res.results[2 * b + 1]["y"]
    return Y
